# revision 14
# baseline (speedup 1.0000x reference)
"""Trainium2 Bass kernel for CrossAttentionWithTemporalEmbedding.

Problem (hardcoded shapes): B=4, C=256, QC=32, H=W=64, HW=4096.
  f1e = f1 + t_emb1; f2e = f2 + t_emb2
  q_i = wq@f_ie + bq; k_i = wk@f_ie + bk; v_i = wv@f_ie + bv   (1x1 convs)
  out1 = g * softmax(q2^T k1) @ v1^T + f1
  out2 = g * softmax(q1^T k2) @ v2^T + f2

Sharding: 8 independent (batch, direction) attention problems -> one per core.

Per-core algorithm (v2: fp8e4 DoubleRow PV):
  q,k [128, HW] bf16     <- projections as in v1 (4x-replicated rows for the
                            K=32 tile_position score matmuls).
  vaugT pairs [128,2,CV] fp8e4 <- f_kv chunks (stationary) @ wvT'; wv' is
                            gamma-scaled and V is mean-centered per channel
                            (host adds the mean back), so fp8 V error shrinks.
  per 512-query block nb:
    S^T [m, n] f32 psum  <- K=32 tile_position matmuls (bf16, as v1)
    E  = exp(S^T - 40) bf16 (ACT; constant shift; bf16 has fp32's range)
    rs[n] = sum_m E      <- 8 "mini" matmuls per group ([128,1]-output,
                            lhsT = E slice, rhs = ones) accumulated in PSUM.
    g[n] = e^4 / rs[n]   <- DVE reciprocal; PE transposes [128,1]->[1,128]
                            and a K=1 ones matmul broadcasts g across
                            partitions -> g_rep [128, 2, 512] bf16.
    E8 = E * g_rep  fp8e4 (Pool/DVE tensor_mul; per-query rescale centers
                            every row's top at e^4 -- exact softmax range
                            management, no row max needed; the scale cancels
                            because the denominator is then the CONSTANT e^4)
    out[c, n] psum       <- DoubleRow fp8 matmuls: lhsT = vaug pair
                            [128,2,128-cc], rhs = E8 pair [128,2,512];
                            2 c-chunks x 16 pairs, ~0.5 cycles/column.
    res = out * e^-4     <- constant "normalization" (gamma folded into wv';
                            per-query g cancels); bf16 -> DMA out[C, hw].
  Host: out += f_kv + mu (V-centering mean), all fp32, free.

Measured building blocks (HW loop-slope): bf16 PV chunk-matmul 137ns;
fp8 DoubleRow pair 105ns; mini 23ns; score group matmul 217ns; Pool rescale
2.13us/group vs DVE 1.03us/group (split tuned so pairs arrive before the
PE consumes them). Expected engine busy: PE ~135us, ACT ~117us, DVE/Pool
~100-115us.
"""

import sys

import numpy as np

sys.path.insert(0, "/opt/trn_rl_repo")

from concourse import bacc, tile, mybir  # noqa: E402
from concourse import bass_utils  # noqa: E402

DT = mybir.dt
AF = mybir.ActivationFunctionType
_bf16 = mybir.dt.np(DT.bfloat16)
_f8 = mybir.dt.np(DT.float8e4)

C = 256
QC = 32
CV = 256          # v columns (no ones/pad column needed in v2)
B = 4
H = W = 64
HW = H * W
NB = 512          # queries per n-block
SCW = 2           # score chunks per group / exp instruction (= fp8 pair)
SHIFT = 40.0      # softmax logit shift (max |logit| ~ 70 < SHIFT + 88)
EM = 4.0          # fp8 window margin: E8 top-of-row lands in [e^-4.3, e^4]
# Schraudolph bit-trick exp constants (scores arrive in PSUM pre-scaled by
# SCH_A via the host-scaled wq; ACT path compensates with scale=1/SCH_A).
SCH_A = 8388608.0 / 0.6931471805599453          # 2^23 / ln 2
SCH_B = 1064986823.0 - SHIFT * SCH_A            # (127<<23) - 366393 - 40*A
DVE_EXP = (5, 10, 15)   # groups/block whose exp runs on the DVE (bit trick)
# rescale engine split: pairs on Pool (slow, 2.13us/group) vs DVE (1.03us),
# balanced against DVE's Schraudolph-exp + norm/g duties.
POOL_RS = (1, 3, 5, 7, 9, 11, 13, 14)

ABLATE = ""   # dev: "nopv" = skip PV+norm; "nors" = also skip rescales;
              # "noex" = scores only (skip exp/minis/g/rescale/pv)
_program_cache = {}

# Set by test harnesses: TRACE=True makes kernel() collect an NTFF profile;
# the BassKernelResults lands in LAST_RESULTS for exec-time/trace inspection.
TRACE = False
LAST_RESULTS = None


def build_program(hw=HW, num_devices=8, reps=1, loop=0):
    key = (hw, num_devices, reps, loop, ABLATE)
    if key in _program_cache:
        return _program_cache[key]

    n_mchunk = hw // 128          # key chunks of 128
    n_pair = n_mchunk // SCW      # fp8 DoubleRow pairs (= score groups)
    n_block = hw // NB            # query blocks of 512
    n_sub = NB // 128             # 128-query sub-slices per block

    nc = bacc.Bacc("TRN2", target_bir_lowering=False, debug=False,
                   num_devices=num_devices)

    fq = nc.dram_tensor("fq", (128, 2, hw), DT.bfloat16, kind="ExternalInput")
    fkv = nc.dram_tensor("fkv", (128, 2, hw), DT.bfloat16,
                         kind="ExternalInput")
    wqkv = nc.dram_tensor("wqkv", (128, 2, 256 + CV), DT.bfloat16,
                          kind="ExternalInput")
    bqk = nc.dram_tensor("bqk", (128, 2), DT.float32, kind="ExternalInput")
    bvf = nc.dram_tensor("bvf", (128, CV), DT.bfloat16, kind="ExternalInput")
    ident = nc.dram_tensor("ident", (128, 128), DT.bfloat16,
                           kind="ExternalInput")
    out = nc.dram_tensor("out", (C, hw), DT.bfloat16, kind="ExternalOutput")

    with tile.TileContext(nc) as tc:
        with (
            tc.tile_pool(name="const", bufs=1) as const,
            tc.tile_pool(name="feat", bufs=1) as feat,
            tc.tile_pool(name="qk", bufs=1) as qkpool,
            tc.tile_pool(name="vaug", bufs=1) as vpool,
            tc.tile_pool(name="ebf", bufs=40) as ebf,
            tc.tile_pool(name="ef8", bufs=18) as ef8,
            tc.tile_pool(name="grep", bufs=2) as grepp,
            tc.tile_pool(name="res", bufs=2) as respool,
            tc.tile_pool(name="small", bufs=8) as small,
            tc.tile_pool(name="ps_sc", bufs=2, space="PSUM") as ps_sc,
            tc.tile_pool(name="ps_pv", bufs=3, space="PSUM") as ps_pv,
            tc.tile_pool(name="ps_aux", bufs=1, space="PSUM") as ps_aux,
        ):
            # ---- constants / weights (batched DMAs on the ACT DGE ring) ----
            wqkv_sb = const.tile([128, 2, 256 + CV], DT.bfloat16)
            bqk_sb = const.tile([128, 2], DT.float32)
            ident_sb = const.tile([128, 128], DT.bfloat16)
            bvf_sb = const.tile([128, CV], DT.bfloat16)
            shift_sb = const.tile([128, 1], DT.float32)
            iscale_sb = const.tile([128, 1], DT.float32)
            ones_mini = const.tile([128, 1], DT.bfloat16)
            ones128 = const.tile([128, 128], DT.bfloat16)
            # zero-padded g rows (row 0 rewritten per block; rows 1-127 stay
            # zero) so g-replication is a K=128 full-array matmul -- the PE
            # never leaves full-array bf16 mode except for the fp8 PV phases.
            gz = [const.tile([128, NB], DT.bfloat16, name=f"gz{i}")
                  for i in range(2)]
            nc.gpsimd.memset(shift_sb[:], -SHIFT)
            nc.gpsimd.memset(iscale_sb[:], 1.0 / SCH_A)
            nc.gpsimd.memset(ones_mini[:], 1.0)
            nc.gpsimd.memset(ones128[:], 1.0)
            nc.gpsimd.memset(gz[0][:], 0.0)
            nc.gpsimd.memset(gz[1][:], 0.0)
            nc.scalar.dma_start(wqkv_sb[:], wqkv.ap()[:])
            nc.scalar.dma_start(bqk_sb[:], bqk.ap()[:])
            nc.scalar.dma_start(bvf_sb[:], bvf.ap()[:])
            nc.scalar.dma_start(ident_sb[:], ident.ap()[:])
            wq_sb = wqkv_sb[:, :, 0:128]
            wk_sb = wqkv_sb[:, :, 128:256]
            wv_sb = wqkv_sb[:, :, 256:256 + CV]
            bq_sb = bqk_sb[:, 0:1]
            bk_sb = bqk_sb[:, 1:2]

            import contextlib
            loop_cm = (tc.For_i(0, loop, 1,
                                hint_engines=(mybir.EngineType.PE,
                                              mybir.EngineType.Activation,
                                              mybir.EngineType.DVE,
                                              mybir.EngineType.Pool,
                                              mybir.EngineType.SP))
                       if loop else contextlib.nullcontext())
            with loop_cm:
              for _rep in range(reps):
                # ---- features: staged in ebf-ring tiles (consumed by the
                # projections during the prologue; the ring then recycles the
                # buffers as E tiles). DMA rides the ACT DGE ring.
                FW = SCW * NB            # fkv staging piece width
                fkv_t = [[ebf.tile([128, FW], DT.bfloat16, tag="e",
                                   name=f"fkvt{kc}_{j}")
                          for j in range(hw // FW)] for kc in range(2)]
                for kc in range(2):
                    for j in range(hw // FW):
                        nc.scalar.dma_start(
                            fkv_t[kc][j][:],
                            fkv.ap()[:, kc, j * FW:(j + 1) * FW])

                def fkv_block(kc, nb):       # [128, NB] slice for block nb
                    j, r = divmod(nb, FW // NB)
                    return fkv_t[kc][j][:, r * NB:(r + 1) * NB]

                def fkv_chunk(kc, m):        # [128, 128] key-chunk slice
                    j, r = divmod(m, FW // 128)
                    return fkv_t[kc][j][:, r * 128:(r + 1) * 128]

                # HAM warm-keepers: keep the PE activity window busy across
                # the loop-boundary DMA wait so the clock gate stays at max.
                warm = ps_pv.tile([128, NB], DT.float32, tag="pvps",
                                  name="warm0")
                nc.tensor.matmul(warm[:, 0:256], wqkv_sb[:, 0, 0:128],
                                 wqkv_sb[:, 0, 0:256], start=True, stop=True)
                warm2 = ps_pv.tile([128, NB], DT.float32, tag="pvps",
                                   name="warm1")
                nc.tensor.matmul(warm2[:, 0:256], fkv_t[0][0][:, 0:128],
                                 fkv_t[0][0][:, 0:256], start=True, stop=True)

                # ---- projections: q,k replicated 4x over partition groups
                q_sb = qkpool.tile([128, hw], DT.bfloat16, tag="q")
                k_sb = qkpool.tile([128, hw], DT.bfloat16, tag="k")

                def proj_psums(nbs, label):
                    tiles = [ps_sc.tile([128, SCW, NB], DT.float32, tag="sc",
                                        name=f"{label}ps{nb}")
                             for nb in nbs[::SCW]]
                    return [tiles[i // SCW][:, i % SCW, :]
                            for i in range(len(nbs))]

                def q_group(nbg):
                    nbs = list(range(nbg, min(nbg + 4, n_block)))
                    fqcs = []
                    for nb in nbs:
                        nsl = slice(nb * NB, (nb + 1) * NB)
                        fqc = feat.tile([128, 2, NB], DT.bfloat16,
                                        tag="fqc", bufs=4, name=f"fqc{nb}")
                        nc.sync.dma_start(fqc[:], fq.ap()[:, :, nsl])
                        fqcs.append(fqc)
                    pss = proj_psums(nbs, "q")
                    for kc in range(2):
                        for i, nb in enumerate(nbs):
                            nc.tensor.matmul(
                                pss[i], wq_sb[:, kc, :], fqcs[i][:, kc],
                                start=(kc == 0), stop=(kc == 1),
                            )
                    for i, nb in enumerate(nbs):
                        nc.scalar.add(
                            q_sb[:, nb * NB:(nb + 1) * NB], pss[i],
                            bq_sb[:])

                def k_group(nbg):
                    nbs = list(range(nbg, min(nbg + 4, n_block)))
                    pss = proj_psums(nbs, "k")
                    for kc in range(2):
                        for i, nb in enumerate(nbs):
                            nc.tensor.matmul(
                                pss[i], wk_sb[:, kc, :],
                                fkv_block(kc, nb),
                                start=(kc == 0), stop=(kc == 1),
                            )
                    for i, nb in enumerate(nbs):
                        nc.vector.tensor_scalar_add(
                            k_sb[:, nb * NB:(nb + 1) * NB], pss[i],
                            bk_sb[:])

                q_group(0)
                q_group(4)
                k_group(0)
                k_group(4)

                # ---- vaug pairs: [128, 2, CV] fp8e4, bias rides the evac
                vaug = [None] * n_pair

                def emit_vaug_pair(p):
                    vt = vpool.tile([128, 2, CV], DT.float8e4, tag=f"v{p}",
                                    name=f"vt{p}")
                    for i in range(2):
                        m = 2 * p + i
                        ps = ps_pv.tile([128, NB], DT.float32, tag="pvps",
                                        name=f"vps{m}")
                        for kc in range(2):
                            nc.tensor.matmul(
                                ps[:, 0:CV], fkv_chunk(kc, m),
                                wv_sb[:, kc, :], start=(kc == 0),
                                stop=(kc == 1),
                            )
                        nc.vector.tensor_add(vt[:, i, :], ps[:, 0:CV],
                                             bvf_sb[:])
                    vaug[p] = vt

                # ---- per-block score/exp/mini/g helpers ----
                et_bf = [[None] * n_pair for _ in range(n_block)]
                rs_ps = [None] * n_block
                rs_r = [None] * n_block
                gt_ps = [None] * n_block
                gt_sb = [None] * n_block

                def emit_sc_group(nb, mg):
                    """Scores for m-chunks [2mg, 2mg+1] of block nb -> exp."""
                    nsl = slice(nb * NB, (nb + 1) * NB)
                    sps = ps_sc.tile([128, SCW, NB], DT.float32, tag="sc",
                                     name=f"sps{nb}_{mg}")
                    for mi in range(SCW):
                        m = mg * SCW + mi
                        nc.tensor.matmul(
                            sps[:, mi, :],
                            k_sb[:, m * 128:(m + 1) * 128],
                            q_sb[:, nsl],
                            start=True, stop=True,
                        )
                    et = ebf.tile([128, SCW, NB], DT.bfloat16, tag="e",
                                  name=f"et{nb}_{mg}")
                    if ABLATE == "noex":
                        nc.vector.tensor_copy(et[:], sps[:])
                    elif mg in DVE_EXP:
                        # Schraudolph exp on the DVE: e^x ~= bitcast_f32(
                        # int32(A*x + B)); A*s is already in PSUM (wq host-
                        # scaled), clamp at 0 handles x < -87 underflow.
                        si = ebf.tile([128, SCW, NB], DT.int32, tag="di",
                                      bufs=3, name=f"di{nb}_{mg}")
                        nc.vector.tensor_scalar(
                            si[:], sps[:], SCH_B, 0.0,
                            mybir.AluOpType.add, mybir.AluOpType.max)
                        nc.vector.tensor_copy(et[:],
                                              si.bitcast(DT.float32)[:])
                    else:
                        nc.scalar.activation(et[:], sps[:], AF.Exp,
                                             bias=shift_sb[:],
                                             scale=iscale_sb[:])
                    et_bf[nb][mg] = et
                    return et

                def emit_minis(nb, mg):
                    """Rowsum partials of group mg into rs_ps[nb][:, ns]."""
                    if ABLATE in ("nors", "noex"):
                        return
                    if rs_ps[nb] is None:
                        rs_ps[nb] = ps_aux.tile([128, 4], DT.float32,
                                                tag="aux", name=f"rs{nb}")
                    et = et_bf[nb][mg]
                    for ns in range(n_sub):
                        for mi in range(SCW):
                            nc.tensor.matmul(
                                rs_ps[nb][:, ns:ns + 1],
                                et[:, mi, ns * 128:(ns + 1) * 128],
                                ones_mini[:],
                                start=(mg == 0 and ns == 0 and mi == 0),
                                stop=(mg == n_pair - 1 and ns == n_sub - 1
                                      and mi == SCW - 1),
                            )

                def emit_g_head(nb):
                    if ABLATE in ("nors", "noex"):
                        return
                    """reciprocal + partition->free flip: rs -> gt_sb [1,NB].

                    The flip is a REGULAR matmul with the identity as moving
                    operand: out[0, j] = sum_p rr[p, ns] * I[p, j] = rr[j, ns]
                    -- a column transpose with standard fp32 psum accumulate
                    semantics (start only on the first, so the 2KB pending-
                    zero region covers all four 128-wide segments)."""
                    rr_f = small.tile([128, 4], DT.float32, tag="rrf",
                                      name=f"rrf{nb}")
                    nc.vector.reciprocal(rr_f[:], rs_ps[nb][:])
                    rr = small.tile([128, 4], DT.bfloat16, tag="rr",
                                    name=f"rr{nb}")
                    nc.vector.tensor_scalar_mul(rr[:], rr_f[:],
                                                float(np.exp(EM)))
                    rs_r[nb] = rr
                    gt = ps_aux.tile([1, NB], DT.float32, tag="aux",
                                     name=f"gt{nb}")
                    for ns in range(n_sub):
                        nc.tensor.matmul(
                            gt[0:1, ns * 128:(ns + 1) * 128],
                            rr[:, ns:ns + 1], ident_sb[:],
                            start=(ns == 0), stop=(ns == n_sub - 1),
                        )
                    gt_ps[nb] = gt
                    gs = gz[nb % 2]
                    nc.vector.tensor_copy(gs[0:1, :], gt[:])
                    gt_sb[nb] = gs

                def emit_g_rep(nb):
                    if ABLATE in ("nors", "noex"):
                        return None
                    """Broadcast gt_sb[nb] across partitions -> [128,2,NB]."""
                    g_rep = grepp.tile([128, 2, NB], DT.bfloat16, tag="grep",
                                       name=f"grep{nb}")
                    for mi in range(2):
                        gps = ps_pv.tile([128, NB], DT.float32, tag="pvps",
                                         name=f"gps{nb}_{mi}")
                        nc.tensor.matmul(gps[:], ones128[:], gt_sb[nb][:],
                                         start=True, stop=True)
                        nc.vector.tensor_copy(g_rep[:, mi, :], gps[:])
                    return g_rep

                def emit_rescale(nb, mg, g_rep):
                    if ABLATE in ("nors", "noex"):
                        return None
                    et8 = ef8.tile([128, SCW, NB], DT.float8e4, tag="e8",
                                   name=f"et8_{nb}_{mg}")
                    eng = nc.gpsimd if mg in POOL_RS else nc.vector
                    eng.tensor_mul(et8[:], et_bf[nb][mg][:], g_rep[:])
                    return et8

                # ---- prologue: vaug + scores/exp/minis for blocks 0,1;
                # rescales for block 0. The main loop then runs a depth-2
                # software pipeline: in span nb it emits PV(nb) (fully
                # supplied: all of block nb's fp8 tiles were rescaled during
                # span nb-1), rescales(nb+1), scores/exp/minis(nb+2), and the
                # g-chain(nb+2) at the tail. The decoupled PV work gives the
                # in-order PE queue ready filler behind every score matmul
                # that waits on exp's psum drain, keeping the PE dense (HAM
                # clock-gate) without more score-psum banks.
                e8 = [[None] * n_pair for _ in range(n_block)]
                g_reps = [None] * n_block
                for p in range(n_pair):
                    emit_vaug_pair(p)
                    emit_sc_group(0, p)
                    emit_minis(0, p)
                emit_g_head(0)
                g_reps[0] = emit_g_rep(0)
                for mg in range(n_pair):
                    e8[0][mg] = emit_rescale(0, mg, g_reps[0])
                    emit_sc_group(1, mg)
                    if mg >= 2:
                        emit_minis(1, mg - 2)
                emit_minis(1, n_pair - 2)
                emit_minis(1, n_pair - 1)
                emit_g_head(1)
                g_reps[1] = emit_g_rep(1)

                # ---- main block loop (span nb) ----
                for nb in range(n_block):
                    pv = [None, None]
                    rblk = respool.tile([128, 2, NB], DT.bfloat16,
                                        tag="res", name=f"rblk{nb}")

                    def pv_pair(p, nb=nb, pv=pv):
                        if ABLATE in ("nopv", "nors", "noex"):
                            return
                        for cc in range(2):
                            if pv[cc] is None:
                                pv[cc] = ps_pv.tile([128, NB], DT.float32,
                                                    tag="pvps",
                                                    name=f"pv{nb}_{cc}")
                            nc.tensor.matmul(
                                pv[cc][:],
                                vaug[p][:, :, cc * 128:(cc + 1) * 128],
                                e8[nb][p][:],
                                start=(p == 0), stop=(p == n_pair - 1),
                                perf_mode=mybir.MatmulPerfMode.DoubleRow,
                            )

                    # quarter phases: [sc,sc][pv x4][sc,sc][minis x4] --
                    # the fp8 PV burst (fully supplied a span ahead) and the
                    # bf16 work are clustered so the PE only changes array
                    # mode ~8x per block (mode switches drain the PE,
                    # ~0.3-0.6us each), while each phase stays under the
                    # 2-group exp psum leash (~2.5us).
                    for qd in range(4):
                        for mg in (4 * qd, 4 * qd + 1):
                            if nb + 1 < n_block:
                                e8[nb + 1][mg] = emit_rescale(
                                    nb + 1, mg, g_reps[nb + 1])
                            if nb + 2 < n_block:
                                emit_sc_group(nb + 2, mg)
                        for p in range(4 * qd, 4 * qd + 4):
                            pv_pair(p)
                        for mg in (4 * qd + 2, 4 * qd + 3):
                            if nb + 1 < n_block:
                                e8[nb + 1][mg] = emit_rescale(
                                    nb + 1, mg, g_reps[nb + 1])
                            if nb + 2 < n_block:
                                emit_sc_group(nb + 2, mg)
                        if nb + 2 < n_block and qd >= 1:
                            for mg in range(4 * (qd - 1), 4 * qd):
                                emit_minis(nb + 2, mg)
                    if nb + 2 < n_block:
                        for mg in range(4 * (4 - 1), n_pair):
                            emit_minis(nb + 2, mg)
                        emit_g_head(nb + 2)
                        g_reps[nb + 2] = emit_g_rep(nb + 2)

                    # constant normalization + writeback ([C, hw] layout)
                    for cc in range(2):
                        if pv[cc] is None:
                            nc.vector.tensor_copy(rblk[:, cc, :],
                                                  et_bf[nb][cc][:, 0, :])
                            continue
                        nc.vector.tensor_scalar_mul(rblk[:, cc, :], pv[cc][:],
                                                    float(np.exp(-EM)))
                    nsl = slice(nb * NB, (nb + 1) * NB)
                    nc.sync.dma_start(
                        out.ap()[:, nsl].rearrange("(a p) n -> p a n", p=128),
                        rblk[:])

    nc.compile()
    _program_cache[key] = nc
    return nc


def _pack_core_inputs(f_q, f_kv, t_q, t_kv, wq, bq, wk, bk, wv, bv, gamma,
                      hw):
    """Host-side packing for one core. f_q/f_kv: [C, hw] fp32.

    Returns the input dict; "_mu" holds the host-side V-centering mean
    (absorbed channel mean of gamma*v, added back to the residual), which
    the device never sees (extra keys are ignored by the runners).
    """
    # q/k ship 4x-replicated along partitions; scores contract all 128 rows
    # (full-array mode, no PE tile switching), so each replica carries q/2,
    # k/2 and the contraction yields 4*(q/2)*(k/2) = q*k exactly.
    bq_eff = (0.5 * SCH_A * (wq @ t_q + bq)).astype(
        np.float32).reshape(QC, 1)
    bk_eff = (0.5 * (wk @ t_kv + bk)).astype(np.float32).reshape(QC, 1)
    wvg = (gamma * wv).astype(np.float32)
    bv_eff = (gamma * (wv @ t_kv + bv)).astype(np.float32)
    # mirror the device's bf16 product to center what it actually computes
    wvg_b = wvg.astype(_bf16).astype(np.float32)
    fkv_b = f_kv.astype(_bf16).astype(np.float32)
    mu = (wvg_b @ fkv_b).mean(axis=1) + bv_eff       # [C]
    return {
        "fq": np.ascontiguousarray(
            f_q.reshape(2, 128, hw).transpose(1, 0, 2)).astype(_bf16),
        "fkv": np.ascontiguousarray(
            f_kv.reshape(2, 128, hw).transpose(1, 0, 2)).astype(_bf16),
        "wqkv": np.concatenate([
            np.tile(0.5 * SCH_A * wq.T, (1, 4)).reshape(2, 128, 128)
            .transpose(1, 0, 2),
            np.tile(0.5 * wk.T, (1, 4)).reshape(2, 128, 128)
            .transpose(1, 0, 2),
            wvg.T.reshape(2, 128, CV).transpose(1, 0, 2),
        ], axis=2).astype(_bf16),
        "bqk": np.concatenate(
            [np.tile(bq_eff, (4, 1)), np.tile(bk_eff, (4, 1))], axis=1),
        "bvf": np.tile((bv_eff - mu).astype(_bf16).reshape(1, CV), (128, 1)),
        "ident": np.eye(128, dtype=_bf16),
        "_mu": mu.astype(np.float32),
    }


def kernel(f1, f2, t_emb1, t_emb2, wq, bq, wk, bk, wv, bv, gamma):
    f1 = np.asarray(f1, np.float32)
    f2 = np.asarray(f2, np.float32)
    t1 = np.asarray(t_emb1, np.float32).ravel()
    t2 = np.asarray(t_emb2, np.float32).ravel()
    wq = np.asarray(wq, np.float32)
    bq = np.asarray(bq, np.float32)
    wk = np.asarray(wk, np.float32)
    bk = np.asarray(bk, np.float32)
    wv = np.asarray(wv, np.float32)
    bv = np.asarray(bv, np.float32)
    g = float(np.asarray(gamma).ravel()[0])
    if g == 0.0:   # attention term vanishes
        return f1.copy(), f2.copy()

    nc = build_program(HW, 8)
    in_maps = []
    mus = []
    for core in range(8):
        d, b = divmod(core, 4)
        if d == 0:   # out1: q from f2, k/v/residual from f1
            f_q, f_kv, t_q, t_kv = f2[b], f1[b], t2, t1
        else:        # out2: q from f1, k/v/residual from f2
            f_q, f_kv, t_q, t_kv = f1[b], f2[b], t1, t2
        m = _pack_core_inputs(
            f_q.reshape(C, HW), f_kv.reshape(C, HW), t_q, t_kv,
            wq, bq, wk, bk, wv, bv, g, HW)
        mus.append(m.pop("_mu"))
        in_maps.append(m)

    global LAST_RESULTS
    res = None
    for attempt in range(3):
        try:
            res = bass_utils.run_bass_kernel_spmd(
                nc, in_maps, core_ids=list(range(8)), trace=TRACE)
            break
        except Exception:
            # First execution after a fresh NEFF compile occasionally hits a
            # transient NRT_EXEC_UNIT_UNRECOVERABLE; a retry succeeds.
            if attempt == 2:
                raise
            import time as _time
            _time.sleep(2.0)
    LAST_RESULTS = res
    o1 = np.empty((B, C, H, W), np.float32)
    o2 = np.empty((B, C, H, W), np.float32)
    for core in range(8):
        d, b = divmod(core, 4)
        f_res = (f1 if d == 0 else f2)[b].reshape(C, HW)
        o = (res.results[core]["out"].astype(np.float32)
             + f_res + mus[core][:, None]).reshape(C, H, W)
        (o1 if d == 0 else o2)[b] = o
    return o1, o2


# revision 17
# speedup vs baseline: 1.2128x; 1.2128x over previous
"""Trainium2 Bass kernel for CrossAttentionWithTemporalEmbedding.

Problem (hardcoded shapes): B=4, C=256, QC=32, H=W=64, HW=4096.
  f1e = f1 + t_emb1; f2e = f2 + t_emb2
  q_i = wq@f_ie + bq; k_i = wk@f_ie + bk; v_i = wv@f_ie + bv   (1x1 convs)
  out1 = g * softmax(q2^T k1) @ v1^T + f1
  out2 = g * softmax(q1^T k2) @ v2^T + f2

Sharding: 8 independent (batch, direction) attention problems -> one per core.

Per-core algorithm (v2: fp8e4 DoubleRow PV):
  q,k [128, HW] bf16     <- projections as in v1 (4x-replicated rows for the
                            K=32 tile_position score matmuls).
  vaugT pairs [128,2,CV] fp8e4 <- f_kv chunks (stationary) @ wvT'; wv' is
                            gamma-scaled and V is mean-centered per channel
                            (host adds the mean back), so fp8 V error shrinks.
  per 512-query block nb:
    S^T [m, n] f32 psum  <- K=32 tile_position matmuls (bf16, as v1)
    E  = exp(S^T - 40) bf16 (ACT; constant shift; bf16 has fp32's range)
    rs[n] = sum_m E      <- 8 "mini" matmuls per group ([128,1]-output,
                            lhsT = E slice, rhs = ones) accumulated in PSUM.
    g[n] = e^4 / rs[n]   <- DVE reciprocal; PE transposes [128,1]->[1,128]
                            and a K=1 ones matmul broadcasts g across
                            partitions -> g_rep [128, 2, 512] bf16.
    E8 = E * g_rep  fp8e4 (Pool/DVE tensor_mul; per-query rescale centers
                            every row's top at e^4 -- exact softmax range
                            management, no row max needed; the scale cancels
                            because the denominator is then the CONSTANT e^4)
    out[c, n] psum       <- DoubleRow fp8 matmuls: lhsT = vaug pair
                            [128,2,128-cc], rhs = E8 pair [128,2,512];
                            2 c-chunks x 16 pairs, ~0.5 cycles/column.
    res = out * e^-4     <- constant "normalization" (gamma folded into wv';
                            per-query g cancels); bf16 -> DMA out[C, hw].
  Host: out += f_kv + mu (V-centering mean), all fp32, free.

Measured building blocks (HW loop-slope): bf16 PV chunk-matmul 137ns;
fp8 DoubleRow pair 105ns; mini 23ns; score group matmul 217ns; Pool rescale
2.13us/group vs DVE 1.03us/group (split tuned so pairs arrive before the
PE consumes them). Expected engine busy: PE ~135us, ACT ~117us, DVE/Pool
~100-115us.
"""

import sys

import numpy as np

sys.path.insert(0, "/opt/trn_rl_repo")

from concourse import bacc, tile, mybir  # noqa: E402
from concourse import bass_utils  # noqa: E402

DT = mybir.dt
AF = mybir.ActivationFunctionType
_bf16 = mybir.dt.np(DT.bfloat16)
_f8 = mybir.dt.np(DT.float8e4)

C = 256
QC = 32
CV = 256          # v columns (no ones/pad column needed in v2)
B = 4
H = W = 64
HW = H * W
NB = 512          # queries per n-block
SCW = 2           # score chunks per group / exp instruction (= fp8 pair)
SHIFT = 40.0      # softmax logit shift (max |logit| ~ 70 < SHIFT + 88)
EM = 4.0          # fp8 window margin: E8 top-of-row lands in [e^-4.3, e^4]
# Schraudolph bit-trick exp constants (scores arrive in PSUM pre-scaled by
# SCH_A via the host-scaled wq; ACT path compensates with scale=1/SCH_A).
SCH_A = 8388608.0 / 0.6931471805599453          # 2^23 / ln 2
SCH_B = 1064986823.0 - SHIFT * SCH_A            # (127<<23) - 366393 - 40*A
DVE_EXP = ()            # groups/block whose exp runs on the DVE (bit trick)
# rescale engine split: pairs on Pool (slow, 2.13us/group) vs DVE (1.03us),
# balanced against DVE's Schraudolph-exp + norm/g duties.
POOL_RS = (2, 5, 8, 11, 13, 15)

ABLATE = ""   # dev: "nopv" = skip PV+norm; "nors" = also skip rescales;
              # "noex" = scores only (skip exp/minis/g/rescale/pv)
_program_cache = {}

# Set by test harnesses: TRACE=True makes kernel() collect an NTFF profile;
# the BassKernelResults lands in LAST_RESULTS for exec-time/trace inspection.
TRACE = False
LAST_RESULTS = None


def build_program(hw=HW, num_devices=8, reps=1, loop=0):
    key = (hw, num_devices, reps, loop, ABLATE)
    if key in _program_cache:
        return _program_cache[key]

    n_mchunk = hw // 128          # key chunks of 128
    n_pair = n_mchunk // SCW      # fp8 DoubleRow pairs (= score groups)
    n_block = hw // NB            # query blocks of 512
    n_sub = NB // 128             # 128-query sub-slices per block

    nc = bacc.Bacc("TRN2", target_bir_lowering=False, debug=False,
                   num_devices=num_devices)

    fq = nc.dram_tensor("fq", (128, 2, hw), DT.bfloat16, kind="ExternalInput")
    fkv = nc.dram_tensor("fkv", (128, 2, hw), DT.bfloat16,
                         kind="ExternalInput")
    wqkv = nc.dram_tensor("wqkv", (128, 2, 256 + CV), DT.bfloat16,
                          kind="ExternalInput")
    bqk = nc.dram_tensor("bqk", (128, 2), DT.float32, kind="ExternalInput")
    bvf = nc.dram_tensor("bvf", (128, CV), DT.bfloat16, kind="ExternalInput")
    ident = nc.dram_tensor("ident", (128, 128), DT.bfloat16,
                           kind="ExternalInput")
    out = nc.dram_tensor("out", (C, hw), DT.bfloat16, kind="ExternalOutput")

    with tile.TileContext(nc) as tc:
        with (
            tc.tile_pool(name="const", bufs=1) as const,
            tc.tile_pool(name="feat", bufs=1) as feat,
            tc.tile_pool(name="qk", bufs=1) as qkpool,
            tc.tile_pool(name="vaug", bufs=1) as vpool,
            tc.tile_pool(name="ebf", bufs=52) as ebf,
            tc.tile_pool(name="ef8", bufs=34) as ef8,
            tc.tile_pool(name="grep", bufs=2) as grepp,
            tc.tile_pool(name="res", bufs=2) as respool,
            tc.tile_pool(name="small", bufs=8) as small,
            tc.tile_pool(name="ps_sc", bufs=2, space="PSUM") as ps_sc,
            tc.tile_pool(name="ps_pv", bufs=3, space="PSUM") as ps_pv,
            tc.tile_pool(name="ps_aux", bufs=1, space="PSUM") as ps_aux,
        ):
            # ---- constants / weights (batched DMAs on the ACT DGE ring) ----
            wqkv_sb = const.tile([128, 2, 256 + CV], DT.bfloat16)
            bqk_sb = const.tile([128, 2], DT.float32)
            ident_sb = const.tile([128, 128], DT.bfloat16)
            bvf_sb = const.tile([128, CV], DT.bfloat16)
            shift_sb = const.tile([128, 1], DT.float32)
            iscale_sb = const.tile([128, 1], DT.float32)
            ones_mini = const.tile([128, 1], DT.bfloat16)
            ones128 = const.tile([128, 128], DT.bfloat16)
            # zero-padded g rows (row 0 rewritten per block; rows 1-127 stay
            # zero) so g-replication is a K=128 full-array matmul -- the PE
            # never leaves full-array bf16 mode except for the fp8 PV phases.
            gz = [const.tile([128, NB], DT.bfloat16, name=f"gz{i}")
                  for i in range(2)]
            nc.gpsimd.memset(shift_sb[:], -SHIFT)
            nc.gpsimd.memset(iscale_sb[:], 1.0 / SCH_A)
            nc.gpsimd.memset(ones_mini[:], 1.0)
            nc.gpsimd.memset(ones128[:], 1.0)
            nc.gpsimd.memset(gz[0][:], 0.0)
            nc.gpsimd.memset(gz[1][:], 0.0)
            nc.scalar.dma_start(wqkv_sb[:], wqkv.ap()[:])
            nc.scalar.dma_start(bqk_sb[:], bqk.ap()[:])
            nc.scalar.dma_start(bvf_sb[:], bvf.ap()[:])
            nc.scalar.dma_start(ident_sb[:], ident.ap()[:])
            wq_sb = wqkv_sb[:, :, 0:128]
            wk_sb = wqkv_sb[:, :, 128:256]
            wv_sb = wqkv_sb[:, :, 256:256 + CV]
            bq_sb = bqk_sb[:, 0:1]
            bk_sb = bqk_sb[:, 1:2]

            import contextlib
            loop_cm = (tc.For_i(0, loop, 1,
                                hint_engines=(mybir.EngineType.PE,
                                              mybir.EngineType.Activation,
                                              mybir.EngineType.DVE,
                                              mybir.EngineType.Pool,
                                              mybir.EngineType.SP))
                       if loop else contextlib.nullcontext())
            with loop_cm:
              for _rep in range(reps):
                # ---- features: staged in ebf-ring tiles (consumed by the
                # projections during the prologue; the ring then recycles the
                # buffers as E tiles). DMA rides the ACT DGE ring.
                FW = SCW * NB            # fkv staging piece width
                fkv_t = [[ebf.tile([128, FW], DT.bfloat16, tag="e",
                                   name=f"fkvt{kc}_{j}")
                          for j in range(hw // FW)] for kc in range(2)]
                for kc in range(2):
                    for j in range(hw // FW):
                        nc.scalar.dma_start(
                            fkv_t[kc][j][:],
                            fkv.ap()[:, kc, j * FW:(j + 1) * FW])

                def fkv_block(kc, nb):       # [128, NB] slice for block nb
                    j, r = divmod(nb, FW // NB)
                    return fkv_t[kc][j][:, r * NB:(r + 1) * NB]

                def fkv_chunk(kc, m):        # [128, 128] key-chunk slice
                    j, r = divmod(m, FW // 128)
                    return fkv_t[kc][j][:, r * 128:(r + 1) * 128]

                # HAM warm-keepers: keep the PE activity window busy across
                # the loop-boundary DMA wait so the clock gate stays at max.
                warm = ps_pv.tile([128, NB], DT.float32, tag="pvps",
                                  name="warm0")
                nc.tensor.matmul(warm[:, 0:256], wqkv_sb[:, 0, 0:128],
                                 wqkv_sb[:, 0, 0:256], start=True, stop=True)
                warm2 = ps_pv.tile([128, NB], DT.float32, tag="pvps",
                                   name="warm1")
                nc.tensor.matmul(warm2[:, 0:256], fkv_t[0][0][:, 0:128],
                                 fkv_t[0][0][:, 0:256], start=True, stop=True)

                # ---- projections: q,k replicated 4x over partition groups
                q_sb = qkpool.tile([128, hw], DT.bfloat16, tag="q")
                k_sb = qkpool.tile([128, hw], DT.bfloat16, tag="k")

                def proj_psums(nbs, label):
                    tiles = [ps_sc.tile([128, SCW, NB], DT.float32, tag="sc",
                                        name=f"{label}ps{nb}")
                             for nb in nbs[::SCW]]
                    return [tiles[i // SCW][:, i % SCW, :]
                            for i in range(len(nbs))]

                def q_group(nbg):
                    nbs = list(range(nbg, min(nbg + 4, n_block)))
                    fqcs = []
                    for nb in nbs:
                        nsl = slice(nb * NB, (nb + 1) * NB)
                        fqc = feat.tile([128, 2, NB], DT.bfloat16,
                                        tag="fqc", bufs=4, name=f"fqc{nb}")
                        nc.sync.dma_start(fqc[:], fq.ap()[:, :, nsl])
                        fqcs.append(fqc)
                    pss = proj_psums(nbs, "q")
                    for kc in range(2):
                        for i, nb in enumerate(nbs):
                            nc.tensor.matmul(
                                pss[i], wq_sb[:, kc, :], fqcs[i][:, kc],
                                start=(kc == 0), stop=(kc == 1),
                            )
                    for i, nb in enumerate(nbs):
                        nc.scalar.add(
                            q_sb[:, nb * NB:(nb + 1) * NB], pss[i],
                            bq_sb[:])

                def k_group(nbg):
                    nbs = list(range(nbg, min(nbg + 4, n_block)))
                    pss = proj_psums(nbs, "k")
                    for kc in range(2):
                        for i, nb in enumerate(nbs):
                            nc.tensor.matmul(
                                pss[i], wk_sb[:, kc, :],
                                fkv_block(kc, nb),
                                start=(kc == 0), stop=(kc == 1),
                            )
                    for i, nb in enumerate(nbs):
                        nc.vector.tensor_scalar_add(
                            k_sb[:, nb * NB:(nb + 1) * NB], pss[i],
                            bk_sb[:])

                q_group(0)
                q_group(4)
                k_group(0)
                k_group(4)

                # ---- vaug pairs: [128, 2, CV] fp8e4, bias rides the evac
                vaug = [None] * n_pair

                def emit_vaug_pair(p):
                    vt = vpool.tile([128, 2, CV], DT.float8e4, tag=f"v{p}",
                                    name=f"vt{p}")
                    for i in range(2):
                        m = 2 * p + i
                        ps = ps_pv.tile([128, NB], DT.float32, tag="pvps",
                                        name=f"vps{m}")
                        for kc in range(2):
                            nc.tensor.matmul(
                                ps[:, 0:CV], fkv_chunk(kc, m),
                                wv_sb[:, kc, :], start=(kc == 0),
                                stop=(kc == 1),
                            )
                        nc.vector.tensor_add(vt[:, i, :], ps[:, 0:CV],
                                             bvf_sb[:])
                    vaug[p] = vt

                # ---- per-block score/exp/mini/g helpers ----
                et_bf = [[None] * n_pair for _ in range(n_block)]
                rs_ps = [None] * n_block
                rs_r = [None] * n_block
                gt_ps = [None] * n_block
                gt_sb = [None] * n_block

                def emit_sc_group(nb, mg):
                    """Scores for m-chunks [2mg, 2mg+1] of block nb -> exp."""
                    nsl = slice(nb * NB, (nb + 1) * NB)
                    sps = ps_sc.tile([128, SCW, NB], DT.float32, tag="sc",
                                     name=f"sps{nb}_{mg}")
                    for mi in range(SCW):
                        m = mg * SCW + mi
                        nc.tensor.matmul(
                            sps[:, mi, :],
                            k_sb[:, m * 128:(m + 1) * 128],
                            q_sb[:, nsl],
                            start=True, stop=True,
                        )
                    et = ebf.tile([128, SCW, NB], DT.bfloat16, tag="e",
                                  name=f"et{nb}_{mg}")
                    if ABLATE == "noex":
                        nc.vector.tensor_copy(et[:], sps[:])
                    elif mg in DVE_EXP:
                        # Schraudolph exp on the DVE: e^x ~= bitcast_f32(
                        # int32(A*x + B)); A*s is already in PSUM (wq host-
                        # scaled), clamp at 0 handles x < -87 underflow.
                        si = ebf.tile([128, SCW, NB], DT.int32, tag="di",
                                      bufs=3, name=f"di{nb}_{mg}")
                        nc.vector.tensor_scalar(
                            si[:], sps[:], SCH_B, 0.0,
                            mybir.AluOpType.add, mybir.AluOpType.max)
                        nc.vector.tensor_copy(et[:],
                                              si.bitcast(DT.float32)[:])
                    else:
                        nc.scalar.activation(et[:], sps[:], AF.Exp,
                                             bias=shift_sb[:],
                                             scale=iscale_sb[:])
                    et_bf[nb][mg] = et
                    return et

                def emit_minis(nb, mg):
                    """Rowsum partials of group mg into rs_ps[nb][:, ns]."""
                    if ABLATE in ("nors", "noex"):
                        return
                    if rs_ps[nb] is None:
                        rs_ps[nb] = ps_aux.tile([128, 4], DT.float32,
                                                tag="aux", name=f"rs{nb}")
                    et = et_bf[nb][mg]
                    for ns in range(n_sub):
                        for mi in range(SCW):
                            nc.tensor.matmul(
                                rs_ps[nb][:, ns:ns + 1],
                                et[:, mi, ns * 128:(ns + 1) * 128],
                                ones_mini[:],
                                start=(mg == 0 and ns == 0 and mi == 0),
                                stop=(mg == n_pair - 1 and ns == n_sub - 1
                                      and mi == SCW - 1),
                            )

                def emit_g_head(nb):
                    if ABLATE in ("nors", "noex"):
                        return
                    """reciprocal + partition->free flip: rs -> gt_sb [1,NB].

                    The flip is a REGULAR matmul with the identity as moving
                    operand: out[0, j] = sum_p rr[p, ns] * I[p, j] = rr[j, ns]
                    -- a column transpose with standard fp32 psum accumulate
                    semantics (start only on the first, so the 2KB pending-
                    zero region covers all four 128-wide segments)."""
                    rr_f = small.tile([128, 4], DT.float32, tag="rrf",
                                      name=f"rrf{nb}")
                    nc.vector.reciprocal(rr_f[:], rs_ps[nb][:])
                    rr = small.tile([128, 4], DT.bfloat16, tag="rr",
                                    name=f"rr{nb}")
                    nc.vector.tensor_scalar_mul(rr[:], rr_f[:],
                                                float(np.exp(EM)))
                    rs_r[nb] = rr
                    gt = ps_aux.tile([1, NB], DT.float32, tag="aux",
                                     name=f"gt{nb}")
                    for ns in range(n_sub):
                        nc.tensor.matmul(
                            gt[0:1, ns * 128:(ns + 1) * 128],
                            rr[:, ns:ns + 1], ident_sb[:],
                            start=(ns == 0), stop=(ns == n_sub - 1),
                        )
                    gt_ps[nb] = gt
                    gs = gz[nb % 2]
                    nc.vector.tensor_copy(gs[0:1, :], gt[:])
                    gt_sb[nb] = gs

                def emit_g_rep(nb):
                    if ABLATE in ("nors", "noex"):
                        return None
                    """Broadcast gt_sb[nb] across partitions -> [128,2,NB]."""
                    g_rep = grepp.tile([128, 2, NB], DT.bfloat16, tag="grep",
                                       name=f"grep{nb}")
                    for mi in range(2):
                        gps = ps_pv.tile([128, NB], DT.float32, tag="pvps",
                                         name=f"gps{nb}_{mi}")
                        nc.tensor.matmul(gps[:], ones128[:], gt_sb[nb][:],
                                         start=True, stop=True)
                        nc.vector.tensor_copy(g_rep[:, mi, :], gps[:])
                    return g_rep

                def emit_rescale(nb, mg, g_rep):
                    if ABLATE in ("nors", "noex", "norsc"):
                        return None
                    et8 = ef8.tile([128, SCW, NB], DT.float8e4, tag="e8",
                                   name=f"et8_{nb}_{mg}")
                    eng = nc.gpsimd if mg in POOL_RS else nc.vector
                    eng.tensor_mul(et8[:], et_bf[nb][mg][:], g_rep[:])
                    return et8

                # ---- prologue: vaug + scores/exp/minis for blocks 0,1;
                # rescales for block 0. The main loop then runs a depth-2
                # software pipeline: in span nb it emits PV(nb) (fully
                # supplied: all of block nb's fp8 tiles were rescaled during
                # span nb-1), rescales(nb+1), scores/exp/minis(nb+2), and the
                # g-chain(nb+2) at the tail. The decoupled PV work gives the
                # in-order PE queue ready filler behind every score matmul
                # that waits on exp's psum drain, keeping the PE dense (HAM
                # clock-gate) without more score-psum banks.
                e8 = [[None] * n_pair for _ in range(n_block)]
                g_reps = [None] * n_block
                for p in range(n_pair):
                    emit_vaug_pair(p)
                    emit_sc_group(0, p)
                    emit_minis(0, p)
                emit_g_head(0)
                g_reps[0] = emit_g_rep(0)
                for mg in range(n_pair):
                    e8[0][mg] = emit_rescale(0, mg, g_reps[0])
                    emit_sc_group(1, mg)
                    if mg >= 2:
                        emit_minis(1, mg - 2)
                emit_minis(1, n_pair - 2)
                emit_minis(1, n_pair - 1)
                emit_g_head(1)
                g_reps[1] = emit_g_rep(1)

                # ---- main block loop (span nb) ----
                for nb in range(n_block):
                    pv = [None, None]
                    rblk = respool.tile([128, 2, NB], DT.bfloat16,
                                        tag="res", name=f"rblk{nb}")

                    def pv_pair(p, nb=nb, pv=pv):
                        if ABLATE in ("nopv", "nors", "noex", "norsc"):
                            return
                        for cc in range(2):
                            if pv[cc] is None:
                                pv[cc] = ps_pv.tile([128, NB], DT.float32,
                                                    tag="pvps",
                                                    name=f"pv{nb}_{cc}")
                            nc.tensor.matmul(
                                pv[cc][:],
                                vaug[p][:, :, cc * 128:(cc + 1) * 128],
                                e8[nb][p][:],
                                start=(p == 0), stop=(p == n_pair - 1),
                                perf_mode=mybir.MatmulPerfMode.DoubleRow,
                            )

                    # quarter phases: [sc,sc][pv x4][sc,sc][minis x4] --
                    # the fp8 PV burst (fully supplied a span ahead) and the
                    # bf16 work are clustered so the PE only changes array
                    # mode ~8x per block (mode switches drain the PE,
                    # ~0.3-0.6us each), while each phase stays under the
                    # 2-group exp psum leash (~2.5us).
                    for qd in range(4):
                        for mg in (4 * qd, 4 * qd + 1):
                            if nb + 1 < n_block:
                                e8[nb + 1][mg] = emit_rescale(
                                    nb + 1, mg, g_reps[nb + 1])
                            if nb + 2 < n_block:
                                emit_sc_group(nb + 2, mg)
                        for p in range(4 * qd, 4 * qd + 4):
                            pv_pair(p)
                        for mg in (4 * qd + 2, 4 * qd + 3):
                            if nb + 1 < n_block:
                                e8[nb + 1][mg] = emit_rescale(
                                    nb + 1, mg, g_reps[nb + 1])
                            if nb + 2 < n_block:
                                emit_sc_group(nb + 2, mg)
                        if nb + 2 < n_block:
                            for mg in range(max(0, 4 * qd - 2), 4 * qd + 2):
                                emit_minis(nb + 2, mg)
                    if nb + 2 < n_block:
                        emit_minis(nb + 2, n_pair - 2)
                        emit_minis(nb + 2, n_pair - 1)
                        emit_g_head(nb + 2)
                        g_reps[nb + 2] = emit_g_rep(nb + 2)

                    # constant normalization + writeback ([C, hw] layout)
                    for cc in range(2):
                        if pv[cc] is None:
                            nc.vector.tensor_copy(rblk[:, cc, :],
                                                  et_bf[nb][cc][:, 0, :])
                            continue
                        nc.vector.tensor_scalar_mul(rblk[:, cc, :], pv[cc][:],
                                                    float(np.exp(-EM)))
                    nsl = slice(nb * NB, (nb + 1) * NB)
                    nc.sync.dma_start(
                        out.ap()[:, nsl].rearrange("(a p) n -> p a n", p=128),
                        rblk[:])

    nc.compile()
    _program_cache[key] = nc
    return nc


def _pack_core_inputs(f_q, f_kv, t_q, t_kv, wq, bq, wk, bk, wv, bv, gamma,
                      hw):
    """Host-side packing for one core. f_q/f_kv: [C, hw] fp32.

    Returns the input dict; "_mu" holds the host-side V-centering mean
    (absorbed channel mean of gamma*v, added back to the residual), which
    the device never sees (extra keys are ignored by the runners).
    """
    # q/k ship 4x-replicated along partitions; scores contract all 128 rows
    # (full-array mode, no PE tile switching), so each replica carries q/2,
    # k/2 and the contraction yields 4*(q/2)*(k/2) = q*k exactly.
    bq_eff = (0.5 * SCH_A * (wq @ t_q + bq)).astype(
        np.float32).reshape(QC, 1)
    bk_eff = (0.5 * (wk @ t_kv + bk)).astype(np.float32).reshape(QC, 1)
    wvg = (gamma * wv).astype(np.float32)
    bv_eff = (gamma * (wv @ t_kv + bv)).astype(np.float32)
    # mirror the device's bf16 product to center what it actually computes
    wvg_b = wvg.astype(_bf16).astype(np.float32)
    fkv_b = f_kv.astype(_bf16).astype(np.float32)
    mu = (wvg_b @ fkv_b).mean(axis=1) + bv_eff       # [C]
    return {
        "fq": np.ascontiguousarray(
            f_q.reshape(2, 128, hw).transpose(1, 0, 2)).astype(_bf16),
        "fkv": np.ascontiguousarray(
            f_kv.reshape(2, 128, hw).transpose(1, 0, 2)).astype(_bf16),
        "wqkv": np.concatenate([
            np.tile(0.5 * SCH_A * wq.T, (1, 4)).reshape(2, 128, 128)
            .transpose(1, 0, 2),
            np.tile(0.5 * wk.T, (1, 4)).reshape(2, 128, 128)
            .transpose(1, 0, 2),
            wvg.T.reshape(2, 128, CV).transpose(1, 0, 2),
        ], axis=2).astype(_bf16),
        "bqk": np.concatenate(
            [np.tile(bq_eff, (4, 1)), np.tile(bk_eff, (4, 1))], axis=1),
        "bvf": np.tile((bv_eff - mu).astype(_bf16).reshape(1, CV), (128, 1)),
        "ident": np.eye(128, dtype=_bf16),
        "_mu": mu.astype(np.float32),
    }


def kernel(f1, f2, t_emb1, t_emb2, wq, bq, wk, bk, wv, bv, gamma):
    f1 = np.asarray(f1, np.float32)
    f2 = np.asarray(f2, np.float32)
    t1 = np.asarray(t_emb1, np.float32).ravel()
    t2 = np.asarray(t_emb2, np.float32).ravel()
    wq = np.asarray(wq, np.float32)
    bq = np.asarray(bq, np.float32)
    wk = np.asarray(wk, np.float32)
    bk = np.asarray(bk, np.float32)
    wv = np.asarray(wv, np.float32)
    bv = np.asarray(bv, np.float32)
    g = float(np.asarray(gamma).ravel()[0])
    if g == 0.0:   # attention term vanishes
        return f1.copy(), f2.copy()

    nc = build_program(HW, 8)
    in_maps = []
    mus = []
    for core in range(8):
        d, b = divmod(core, 4)
        if d == 0:   # out1: q from f2, k/v/residual from f1
            f_q, f_kv, t_q, t_kv = f2[b], f1[b], t2, t1
        else:        # out2: q from f1, k/v/residual from f2
            f_q, f_kv, t_q, t_kv = f1[b], f2[b], t1, t2
        m = _pack_core_inputs(
            f_q.reshape(C, HW), f_kv.reshape(C, HW), t_q, t_kv,
            wq, bq, wk, bk, wv, bv, g, HW)
        mus.append(m.pop("_mu"))
        in_maps.append(m)

    global LAST_RESULTS
    res = None
    for attempt in range(3):
        try:
            res = bass_utils.run_bass_kernel_spmd(
                nc, in_maps, core_ids=list(range(8)), trace=TRACE)
            break
        except Exception:
            # First execution after a fresh NEFF compile occasionally hits a
            # transient NRT_EXEC_UNIT_UNRECOVERABLE; a retry succeeds.
            if attempt == 2:
                raise
            import time as _time
            _time.sleep(2.0)
    LAST_RESULTS = res
    o1 = np.empty((B, C, H, W), np.float32)
    o2 = np.empty((B, C, H, W), np.float32)
    for core in range(8):
        d, b = divmod(core, 4)
        f_res = (f1 if d == 0 else f2)[b].reshape(C, HW)
        o = (res.results[core]["out"].astype(np.float32)
             + f_res + mus[core][:, None]).reshape(C, H, W)
        (o1 if d == 0 else o2)[b] = o
    return o1, o2
